# revision 5
# baseline (speedup 1.0000x reference)
"""Bidirectional GRU + dense heads on 8 trn2 NeuronCores.

Sharding: cores 0-3 forward GRU (batch pairs), cores 4-7 backward GRU
(host-side time-reversed input, same device program). Each core:
  phase 1: xg^T = Wi^T x^T + bi  (gates in transposed layout, bf16, SBUF)
  phase 2: 4096-step GRU recurrence, transposed gate layout
  phase 3: direction-local head partial GEMM -> part [PB, T, 512] f32
Host: mean = Pf_m + Pb_m + bm ; J = exp(Pf_v + Pb_v + bv).
"""

import numpy as np
import ml_dtypes

import concourse.bass as bass
import concourse.bacc as bacc
import concourse.mybir as mybir
import concourse.tile as tile
from concourse import bass_utils

B, T, D, H = 8, 4096, 256, 256
PB = 2                 # batches per core
NCORES = 8
BF16 = mybir.dt.bfloat16
F32 = mybir.dt.float32
NP_BF16 = ml_dtypes.bfloat16
AF = mybir.ActivationFunctionType
ALU = mybir.AluOpType

_NC_CACHE = {}


def build_nc(t_steps=T, unroll=32):
    """Build + compile the per-core SPMD program (identical on all cores)."""
    key = (t_steps, unroll)
    if key in _NC_CACHE:
        return _NC_CACHE[key]
    rows = t_steps * PB
    n_mtiles = rows // 128          # phase-3 M tiles
    n_chunks = rows // 512 if rows >= 512 else 1
    chunk = min(512, rows)

    nc = bacc.Bacc("TRN2", target_bir_lowering=False, debug=False,
                   num_devices=NCORES)
    xT_d = nc.dram_tensor("xT", [128, 2, rows], BF16, kind="ExternalInput").ap()
    wi_d = nc.dram_tensor("wi", [128, 2, 768], BF16, kind="ExternalInput").ap()
    wh_d = nc.dram_tensor("wh", [128, 2, 768], BF16, kind="ExternalInput").ap()
    bi_d = nc.dram_tensor("bi", [128, 6], F32, kind="ExternalInput").ap()
    bhn_d = nc.dram_tensor("bhn", [128, 2 * PB], BF16, kind="ExternalInput").ap()
    id_d = nc.dram_tensor("ident", [128, 128], BF16, kind="ExternalInput").ap()
    whd_d = nc.dram_tensor("whead", [128, 2, 512], BF16, kind="ExternalInput").ap()
    part_d = nc.dram_tensor("part", [PB, t_steps, 512], F32,
                            kind="ExternalOutput").ap()

    from contextlib import ExitStack
    with tile.TileContext(nc) as tc, ExitStack() as es:
        # persistent SBUF tensors
        pp = es.enter_context(tc.tile_pool(name="persist", bufs=1))
        xT = pp.tile([128, 2, rows], BF16, name="xT_sb")
        xg = pp.tile([128, 6, rows], BF16, name="xg_sb")
        hs = pp.tile([128, 2, (t_steps + 1) * PB], BF16, name="hs_sb")
        wi = pp.tile([128, 2, 768], BF16, name="wi_sb")
        wh = pp.tile([128, 2, 768], BF16, name="wh_sb")
        bi = pp.tile([128, 6], F32, name="bi_sb")
        bhn = pp.tile([128, 2 * PB], BF16, name="bhn_sb")
        ident = pp.tile([128, 128], BF16, name="id_sb")
        whead = pp.tile([128, 2, 512], BF16, name="whead_sb")

        nc.sync.dma_start(xT[:], xT_d[:])
        nc.sync.dma_start(wi[:], wi_d[:])
        nc.sync.dma_start(wh[:], wh_d[:])
        nc.sync.dma_start(bi[:], bi_d[:])
        nc.sync.dma_start(bhn[:], bhn_d[:])
        nc.sync.dma_start(ident[:], id_d[:])
        nc.sync.dma_start(whead[:], whd_d[:])

        # ---- phase 1: xg[m] = Wi[:, m-block]^T @ x^T + bi, bf16 out ----
        with tc.tile_pool(name="p1psum", bufs=4, space="PSUM") as p1p:
            for m in range(6):
                for c in range(n_chunks):
                    ps = p1p.tile([128, chunk], F32)
                    sl = slice(c * chunk, (c + 1) * chunk)
                    nc.tensor.matmul(ps[:], wi[:, 0, m * 128:(m + 1) * 128],
                                     xT[:, 0, sl], start=True, stop=False)
                    nc.tensor.matmul(ps[:], wi[:, 1, m * 128:(m + 1) * 128],
                                     xT[:, 1, sl], start=False, stop=True)
                    nc.scalar.activation(xg[:, m, sl], ps[:], AF.Identity,
                                         bias=bi[:, m:m + 1])

        # h_0 = 0
        nc.vector.memset(hs[:, :, 0:PB], 0.0)

        # ---- phase 2: recurrence ----
        def gru_step(off):
            """off: dynamic row offset of step t (reads slot t, writes t+1)."""
            cur = bass.ds(off, PB)            # hs slot t  (rows offset)
            nxt = bass.ds(off + PB, PB)       # hs slot t+1
            xsl = bass.ds(off, PB)            # xg rows for step t
            ps = p2p.tile([128, 6 * PB], F32)
            # gi / bhn accumulation (no h dependency -> overlaps prev gates)
            for m in range(4):
                nc.tensor.matmul(ps[:, m * PB:(m + 1) * PB], ident[:],
                                 xg[:, m, xsl], start=(m == 0), stop=False,
                                 skip_group_check=True)
            for uu in range(2):
                nc.tensor.matmul(ps[:, (4 + uu) * PB:(5 + uu) * PB], ident[:],
                                 bhn[:, uu * PB:(uu + 1) * PB], start=False,
                                 stop=False, skip_group_check=True)
            # Wh^T h accumulation
            for m in range(6):
                for uu in range(2):
                    nc.tensor.matmul(ps[:, m * PB:(m + 1) * PB],
                                     wh[:, uu, m * 128:(m + 1) * 128],
                                     hs[:, uu, cur],
                                     start=False, stop=(m == 5 and uu == 1),
                                     skip_group_check=True)
            rz = gp.tile([128, 4 * PB], F32)
            nc.scalar.activation(rz[:], ps[:, 0:4 * PB], AF.Sigmoid)
            tmp = gp.tile([128, 2 * PB], F32)
            nc.vector.tensor_mul(tmp[:], rz[:, 0:2 * PB], ps[:, 4 * PB:6 * PB])
            s2 = gp.tile([128, 2 * PB], F32)
            nc.vector.tensor_add(s2[:], tmp[:], xg[:, 4:6, xsl])
            nn = gp.tile([128, 2 * PB], F32)
            nc.scalar.activation(nn[:], s2[:], AF.Tanh)
            # e1 = z*h, f1 = 1-z  (off critical path)
            e1 = gp.tile([128, 2 * PB], F32)
            nc.vector.tensor_mul(e1[:], rz[:, 2 * PB:4 * PB], hs[:, :, cur])
            f1 = gp.tile([128, 2 * PB], F32)
            nc.vector.tensor_scalar(f1[:], rz[:, 2 * PB:4 * PB], -1.0, 1.0,
                                    ALU.mult, ALU.add)
            g1 = gp.tile([128, 2 * PB], F32)
            nc.vector.tensor_mul(g1[:], f1[:], nn[:])
            nc.vector.tensor_add(hs[:, :, nxt], g1[:], e1[:])

        with tc.tile_pool(name="p2psum", bufs=4, space="PSUM") as p2p, \
             tc.tile_pool(name="gates", bufs=3) as gp:
            n_iter = t_steps // unroll
            if n_iter > 1:
                with tc.For_i(0, n_iter) as iv:
                    for u in range(unroll):
                        gru_step(iv * (unroll * PB) + u * PB)
            else:
                for tt in range(t_steps):
                    gru_step(tt * PB)

        # ---- phase 3: partial head GEMM ----
        with tc.tile_pool(name="p3psum", bufs=4, space="PSUM") as p3p, \
             tc.tile_pool(name="p3st", bufs=3) as stp:
            for k in range(n_mtiles):
                ps = p3p.tile([128, 512], F32)
                hsl = slice(PB + k * 128, PB + (k + 1) * 128)
                for uu in range(2):
                    nc.tensor.matmul(ps[:], hs[:, uu, hsl], whead[:, uu, :],
                                     start=(uu == 0), stop=(uu == 1))
                st = stp.tile([128, 512], F32)
                nc.vector.tensor_copy(st[:], ps[:])
                nc.sync.dma_start(
                    part_d[:, k * 64:(k + 1) * 64, :]
                    .rearrange("b t j -> t b j"), st[:])

    nc.compile()
    _NC_CACHE[key] = nc
    return nc


def _pack_w(w):
    """[256, 768ish] f32 -> [128, 2, cols] bf16 (partition = row % 128)."""
    return np.ascontiguousarray(
        w.reshape(2, 128, w.shape[1]).transpose(1, 0, 2)).astype(NP_BF16)


def make_core_inputs(inputs, Wi_f, bi_f, Wh_f, bhn_f, Wi_b, bi_b, Wh_b, bhn_b,
                     Wm, Wv, t_steps=T):
    """Build the 8 per-core input dicts (host-side prep)."""
    x = np.asarray(inputs)[:, :t_steps, :]
    ident = np.eye(128, dtype=NP_BF16)
    ins = []
    for c in range(NCORES):
        fwd = c < 4
        pair = c % 4
        xp = x[2 * pair:2 * pair + 2]          # [PB, T, D]
        if not fwd:
            xp = xp[:, ::-1, :]
        # xT [128, 2, rows]: [p, u, (t, b)] = xp[b, t, u*128+p]
        xt = np.ascontiguousarray(
            xp.transpose(2, 1, 0)               # [D, T, PB]
            .reshape(2, 128, t_steps, PB)
            .transpose(1, 0, 2, 3)
            .reshape(128, 2, t_steps * PB)).astype(NP_BF16)
        Wi, WhW = (Wi_f, Wh_f) if fwd else (Wi_b, Wh_b)
        bi, bhn = (bi_f, bhn_f) if fwd else (bi_b, bhn_b)
        bi_t = np.ascontiguousarray(np.asarray(bi).reshape(6, 128).T
                                    ).astype(np.float32)
        bhn_t = np.asarray(bhn).reshape(2, 128).T       # [128, 2]
        bhnB = np.repeat(bhn_t[:, :, None], PB, axis=2).reshape(128, 2 * PB)
        half = slice(0, 256) if fwd else slice(256, 512)
        whead = np.concatenate(
            [np.asarray(Wm)[half], np.asarray(Wv)[half]], axis=1)  # [256,512]
        ins.append({
            "xT": xt,
            "wi": _pack_w(np.asarray(Wi)),
            "wh": _pack_w(np.asarray(WhW)),
            "bi": bi_t,
            "bhn": bhnB.astype(NP_BF16),
            "ident": ident,
            "whead": _pack_w(whead),
        })
    return ins


def run_device(ins, t_steps=T, unroll=32):
    nc = build_nc(t_steps, unroll)
    res = bass_utils.run_bass_kernel_spmd(nc, ins, core_ids=list(range(NCORES)))
    return [r["part"] for r in res.results]


def kernel(inputs, Wi_f, bi_f, Wh_f, bhn_f, Wi_b, bi_b, Wh_b, bhn_b,
           Wm, bm, Wv, bv):
    ins = make_core_inputs(inputs, Wi_f, bi_f, Wh_f, bhn_f,
                           Wi_b, bi_b, Wh_b, bhn_b, Wm, Wv)
    parts = run_device(ins)
    Pf = np.concatenate(parts[:4], axis=0)                  # [B, T, 512]
    Pb = np.concatenate([p[:, ::-1, :] for p in parts[4:]], axis=0)
    pre = Pf + Pb
    h = pre[..., :256] + np.asarray(bm)
    J = np.exp(pre[..., 256:] + np.asarray(bv))
    return (J.astype(np.float32), h.astype(np.float32))
